# revision 15
# baseline (speedup 1.0000x reference)
"""Trainium2 Bass kernel for nn_Caps_BN (BatchNorm2d + grouped 1x1 conv).

Reference computation (per full input x of shape (64, 512, 32, 32)):
    mean/var per channel over (N, H, W)  [training-mode biased BN, affine=False]
    xn = (x - mean) * rsqrt(var + eps)
    out[n, (c,o), hw] = sum_i W[c, o, i] * xn[n, (c,i), hw] + bias[(c,o)]

Strategy:
  * Data-parallel over the batch dim: 8 cores x 8 batches each.
  * BN is folded into the conv:  out = W' @ x + bias', where
        W'[c,o,i]  = W[c,o,i] * rsqrt(var[c,i] + eps)
        bias'[c,o] = bias[c,o] - sum_i W'[c,o,i] * mean[c,i]
    so the kernel never materializes xn — a single matmul pass over raw x.
  * Per-channel (sum, sumsq) are computed locally with bn_stats/bn_aggr
    (one DVE pass over resident data), then a 4 KB AllReduce combines them
    across the 8 cores.
  * Channels are processed in 4 groups of 128 (= 4 capsules of D=32); each
    group's weights form a block-diagonal 128x128 lhsT so the TensorEngine
    contracts over the full 128-partition dim.
"""

import sys

if "/opt/trn_rl_repo" not in sys.path:
    sys.path.insert(0, "/opt/trn_rl_repo")

import numpy as np

import concourse.bass as bass
import concourse.bacc as bacc
import concourse.mybir as mybir
import concourse.tile as tile
from concourse.bass_utils import run_bass_kernel_spmd

N_CORES = 8
N_FULL = 64
C, D = 16, 32
CD = C * D  # 512 channels
H = W = 32
HW = H * W  # 1024
NL = N_FULL // N_CORES  # batches per core
G = CD // 128  # channel groups of 128 (= 4 capsules each)
CPG = 128 // D  # capsules per group (4)
FC = 512  # matmul moving-operand chunk (fp32 max / one PSUM bank)
EPS = 1e-5

F32 = mybir.dt.float32
ALU = mybir.AluOpType
ACTF = mybir.ActivationFunctionType


def build_nc(nl: int = NL, n_cores: int = N_CORES, copy_split: int = 2):
    """Build the SPMD Bass program (identical on every core).

    copy_split: every copy_split-th PSUM->SBUF bias-add copy goes to the
    Scalar engine (ACT Identity) instead of DVE; 0 = all on DVE.
    """
    f = nl * HW  # free-dim elements per channel group
    ntot = float(n_cores * nl * HW)  # BN population per channel
    n_chunks = f // FC

    nc = bacc.Bacc(
        "TRN2", target_bir_lowering=False, debug=False, num_devices=n_cores
    )
    x_d = nc.dram_tensor("x_shard", [nl, CD, HW], F32, kind="ExternalInput")
    # lhsT_bd is the host-prepared block-diagonal transposed weight:
    # lhsT_bd[g, cl*D+i, cl*D+o] = weight[g*CPG+cl, o, i], zero off-block.
    w_d = nc.dram_tensor("lhsT_bd", [G, 128, 128], F32, kind="ExternalInput")
    b_d = nc.dram_tensor("bias", [CD], F32, kind="ExternalInput")
    o_d = nc.dram_tensor("out", [nl, CD, HW], F32, kind="ExternalOutput")

    with tile.TileContext(nc) as tc:
        with (
            tc.tile_pool(name="xp", bufs=1) as xp,
            tc.tile_pool(name="wp", bufs=1) as wp,
            tc.tile_pool(name="st", bufs=1) as st,
            tc.tile_pool(name="stage", bufs=2) as sp,
            tc.tile_pool(name="ps", bufs=6, space="PSUM") as pp,
            tc.tile_pool(name="psb", bufs=2, space="PSUM") as ppb,
            tc.tile_pool(name="dram", bufs=1, space="DRAM") as dp,
        ):
            # x viewed as (group, channel-in-group, batch, hw)
            xr = x_d.rearrange("n (g p) f -> g p n f", p=128)
            orr = o_d.rearrange("n (g p) f -> g p n f", p=128)

            # ---- weights: one DMA per group for the block-diag lhsT ----
            lhsT = []
            for j in range(G):
                lt = wp.tile([128, 128], F32, tag=f"lhsT{j}", name=f"lhsT{j}")
                nc.sync.dma_start(out=lt[:, :], in_=w_d[j])
                lhsT.append(lt)

            bias_sb = []
            br = b_d.rearrange("(g p one) -> g p one", p=128, one=1)
            for j in range(G):
                bt = st.tile([128, 1], F32, tag=f"bias{j}", name=f"bias{j}")
                nc.sync.dma_start(out=bt[:, :], in_=br[j])
                bias_sb.append(bt)

            # ---- load x, local BN stats --------------------------------
            xt = []
            spack = st.tile([128, 2 * G], F32, tag="spack", name="spack")
            for j in range(G):
                t = xp.tile([128, f], F32, tag=f"x{j}", name=f"x{j}")
                nc.sync.dma_start(
                    out=t.rearrange("p (n f) -> p n f", n=nl), in_=xr[j]
                )
                xt.append(t)

                st6 = st.tile(
                    [128, (f // FC) * 6], F32, tag=f"st6_{j}", name=f"st6_{j}"
                )
                for k in range(f // FC):
                    nc.vector.bn_stats(
                        out=st6[:, k * 6 : (k + 1) * 6],
                        in_=t[:, k * FC : (k + 1) * FC],
                    )
                mv = st.tile([128, 2], F32, tag=f"mv{j}", name=f"mv{j}")
                nc.vector.bn_aggr(
                    out=mv[:, :], in_=st6.rearrange("p (k s) -> p k s", s=3)
                )
                # local sum / sumsq (population of f elems per channel)
                nc.vector.tensor_scalar_mul(
                    spack[:, j : j + 1], mv[:, 0:1], float(f)
                )
                msq = st.tile([128, 1], F32, tag=f"msq{j}", name=f"msq{j}")
                nc.vector.tensor_tensor(
                    msq[:, :], mv[:, 0:1], mv[:, 0:1], ALU.mult
                )
                nc.vector.tensor_tensor(msq[:, :], msq[:, :], mv[:, 1:2], ALU.add)
                nc.vector.tensor_scalar_mul(
                    spack[:, G + j : G + j + 1], msq[:, :], float(f)
                )

            # ---- AllReduce of (sum, sumsq) across cores ----------------
            cc_in = dp.tile([128, 2 * G], F32, tag="ccin", name="ccin")
            cc_out = dp.tile([128, 2 * G], F32, tag="ccout", name="ccout")
            nc.gpsimd.dma_start(out=cc_in[:, :], in_=spack[:, :])
            nc.gpsimd.collective_compute(
                "AllReduce",
                ALU.add,
                replica_groups=[list(range(n_cores))],
                ins=[cc_in.opt()],
                outs=[cc_out.opt()],
            )
            sg = st.tile([128, 2 * G], F32, tag="sg", name="sg")
            nc.gpsimd.dma_start(out=sg[:, :], in_=cc_out[:, :])

            # ---- fold global stats into weights + bias -----------------
            epst = st.tile([128, 1], F32, tag="epst", name="epst")
            nc.vector.memset(epst[:, :], EPS)
            biasp = []
            for j in range(G):
                mean = st.tile([128, 1], F32, tag=f"gmean{j}", name=f"gmean{j}")
                nc.vector.tensor_scalar_mul(mean[:, :], sg[:, j : j + 1], 1.0 / ntot)
                ex2 = st.tile([128, 1], F32, tag=f"gex2{j}", name=f"gex2{j}")
                nc.vector.tensor_scalar_mul(
                    ex2[:, :], sg[:, G + j : G + j + 1], 1.0 / ntot
                )
                msq = st.tile([128, 1], F32, tag=f"gmsq{j}", name=f"gmsq{j}")
                nc.vector.tensor_tensor(msq[:, :], mean[:, :], mean[:, :], ALU.mult)
                var = st.tile([128, 1], F32, tag=f"gvar{j}", name=f"gvar{j}")
                nc.vector.tensor_tensor(var[:, :], ex2[:, :], msq[:, :], ALU.subtract)
                sd = st.tile([128, 1], F32, tag=f"gsd{j}", name=f"gsd{j}")
                nc.scalar.activation(sd[:, :], var[:, :], ACTF.Sqrt, bias=epst[:, :])
                rs = st.tile([128, 1], F32, tag=f"grs{j}", name=f"grs{j}")
                nc.vector.reciprocal(rs[:, :], sd[:, :])
                # scale lhsT rows by rsqrt(var+eps) of the *input* channel
                nc.vector.tensor_scalar_mul(lhsT[j][:, :], lhsT[j][:, :], rs[:, :])
                nmean = st.tile([128, 1], F32, tag=f"gnm{j}", name=f"gnm{j}")
                nc.vector.tensor_scalar_mul(nmean[:, :], mean[:, :], -1.0)
                # bias' = bias - W' @ mean   (block-diag matmul with K=128)
                pb = ppb.tile([128, 1], F32, tag="pbias", name=f"pbias{j}")
                nc.tensor.matmul(
                    pb[:, :], lhsT[j][:, :], nmean[:, :], start=True, stop=True
                )
                bp = st.tile([128, 1], F32, tag=f"gbp{j}", name=f"gbp{j}")
                nc.vector.tensor_tensor(bp[:, :], pb[:, :], bias_sb[j][:, :], ALU.add)
                biasp.append(bp)

            # ---- main: grouped conv as block-diag matmul ---------------
            half = f // 2  # stage/DMA granularity: half a group
            for j in range(G):
                stg = sp.tile([128, f], F32, tag="stage", name=f"stage{j}")
                for ch in range(n_chunks):
                    ps = pp.tile([128, FC], F32, tag="ps", name=f"ps{j}_{ch}")
                    nc.tensor.matmul(
                        ps[:, :],
                        lhsT[j][:, :],
                        xt[j][:, ch * FC : (ch + 1) * FC],
                        start=True,
                        stop=True,
                    )
                    if copy_split and (ch % copy_split == copy_split - 1):
                        nc.scalar.activation(
                            stg[:, ch * FC : (ch + 1) * FC],
                            ps[:, :],
                            ACTF.Identity,
                            bias=biasp[j][:, :],
                        )
                    else:
                        nc.vector.tensor_scalar_add(
                            stg[:, ch * FC : (ch + 1) * FC], ps[:, :], biasp[j][:, :]
                        )
                # store both halves as separate DMAs so draining overlaps
                orj = orr[j]  # (128, nl, HW)
                sgr = stg.rearrange("p (n f) -> p n f", n=nl)
                if nl >= 2:
                    nc.sync.dma_start(out=orj[:, : nl // 2, :], in_=sgr[:, : nl // 2, :])
                    nc.sync.dma_start(out=orj[:, nl // 2 :, :], in_=sgr[:, nl // 2 :, :])
                else:
                    nc.sync.dma_start(out=orj, in_=sgr)

    nc.compile()
    return nc


_NC_CACHE: dict = {}


def _get_nc(nl: int, n_cores: int):
    key = (nl, n_cores)
    if key not in _NC_CACHE:
        _NC_CACHE[key] = build_nc(nl=nl, n_cores=n_cores)
    return _NC_CACHE[key]


def make_lhsT_bd(weight: np.ndarray) -> np.ndarray:
    lb = np.zeros((G, 128, 128), dtype=np.float32)
    for c in range(C):
        g, cl = divmod(c, CPG)
        s = cl * D
        lb[g, s : s + D, s : s + D] = weight[c].T  # (i, o)
    return lb


def make_in_maps(x, weight, bias):
    lhsT_bd = make_lhsT_bd(weight)
    return [
        {
            "x_shard": np.ascontiguousarray(
                x[i * NL : (i + 1) * NL].reshape(NL, CD, HW)
            ),
            "lhsT_bd": lhsT_bd,
            "bias": np.ascontiguousarray(bias),
        }
        for i in range(N_CORES)
    ]


def kernel(x: np.ndarray, weight: np.ndarray, bias: np.ndarray) -> np.ndarray:
    assert x.shape == (N_FULL, CD, H, W) and x.dtype == np.float32
    nc = _get_nc(NL, N_CORES)
    in_maps = make_in_maps(x, weight, bias)
    res = run_bass_kernel_spmd(nc, in_maps, core_ids=list(range(N_CORES)))
    out = np.concatenate(
        [res.results[i]["out"].reshape(NL, CD, H, W) for i in range(N_CORES)], axis=0
    )
    return out.astype(np.float32, copy=False)


# revision 23
# speedup vs baseline: 1.1314x; 1.1314x over previous
"""Trainium2 Bass kernel for nn_Caps_BN (BatchNorm2d + grouped 1x1 conv).

Reference computation (per full input x of shape (64, 512, 32, 32)):
    mean/var per channel over (N, H, W)  [training-mode biased BN, affine=False]
    xn = (x - mean) * rsqrt(var + eps)
    out[n, (c,o), hw] = sum_i W[c, o, i] * xn[n, (c,i), hw] + bias[(c,o)]

Strategy:
  * Data-parallel over the batch dim: 8 cores x 8 batches each.
  * BN is folded into the conv:  out = W' @ x + bias', where
        W'[c,o,i]  = W[c,o,i] * rsqrt(var[c,i] + eps)
        bias'[c,o] = bias[c,o] - sum_i W'[c,o,i] * mean[c,i]
    so the kernel never materializes xn — a single matmul pass over raw x.
  * Per-channel (sum, sumsq) are computed locally with bn_stats/bn_aggr
    (one DVE pass over resident data), then a 4 KB AllReduce combines them
    across the 8 cores.
  * Channels are processed in 4 groups of 128 (= 4 capsules of D=32); each
    group's weights form a block-diagonal 128x128 lhsT so the TensorEngine
    contracts over the full 128-partition dim.
"""

import sys

if "/opt/trn_rl_repo" not in sys.path:
    sys.path.insert(0, "/opt/trn_rl_repo")

import numpy as np

import concourse.bass as bass
import concourse.bacc as bacc
import concourse.mybir as mybir
import concourse.tile as tile
from concourse.bass_utils import run_bass_kernel_spmd

N_CORES = 8
N_FULL = 64
C, D = 16, 32
CD = C * D  # 512 channels
H = W = 32
HW = H * W  # 1024
NL = N_FULL // N_CORES  # batches per core
G = CD // 128  # channel groups of 128 (= 4 capsules each)
CPG = 128 // D  # capsules per group (4)
FC = 512  # matmul moving-operand chunk (fp32 max / one PSUM bank)
EPS = 1e-5

F32 = mybir.dt.float32
ALU = mybir.AluOpType
ACTF = mybir.ActivationFunctionType

# Matmul compute dtype for the main conv loop. float32r streams fp32
# operands through the PE in a single pass (4x the fp32 rate at FD>=256)
# with reduced-precision multiplies; float32 is the exact 2-pass mode.
MM_DTYPE = mybir.dt.float32r


def build_nc(nl: int = NL, n_cores: int = N_CORES, copy_split: int = 2):
    """Build the SPMD Bass program (identical on every core).

    copy_split: every copy_split-th PSUM->SBUF bias-add copy goes to the
    Scalar engine (ACT Identity) instead of DVE; 0 = all on DVE.
    """
    f = nl * HW  # free-dim elements per channel group
    ntot = float(n_cores * nl * HW)  # BN population per channel
    n_chunks = f // FC

    nc = bacc.Bacc(
        "TRN2", target_bir_lowering=False, debug=False, num_devices=n_cores
    )
    # x and the folded weight are typed as the matmul compute dtype
    # (float32r = same 4-byte fp32 bits, single-pass PE mode); all
    # non-matmul consumers view them through .bitcast(F32).
    x_d = nc.dram_tensor("x_shard", [nl, CD, HW], MM_DTYPE, kind="ExternalInput")
    # lhsT_bd is the host-prepared block-diagonal transposed weight:
    # lhsT_bd[g, cl*D+i, cl*D+o] = weight[g*CPG+cl, o, i], zero off-block.
    w_d = nc.dram_tensor("lhsT_bd", [G, 128, 128], MM_DTYPE, kind="ExternalInput")
    b_d = nc.dram_tensor("bias", [CD], F32, kind="ExternalInput")
    o_d = nc.dram_tensor("out", [nl, CD, HW], F32, kind="ExternalOutput")

    with tile.TileContext(nc) as tc:
        with (
            tc.tile_pool(name="xp", bufs=1) as xp,
            tc.tile_pool(name="wp", bufs=1) as wp,
            tc.tile_pool(name="st", bufs=1) as st,
            tc.tile_pool(name="stage", bufs=2) as sp,
            tc.tile_pool(name="ps", bufs=6, space="PSUM") as pp,
            tc.tile_pool(name="psb", bufs=2, space="PSUM") as ppb,
            tc.tile_pool(name="dram", bufs=1, space="DRAM") as dp,
        ):
            # x viewed as (group, channel-in-group, batch, hw)
            xr = x_d.rearrange("n (g p) f -> g p n f", p=128)
            orr = o_d.rearrange("n (g p) f -> g p n f", p=128)

            # ---- weights: one DMA per group for the block-diag lhsT ----
            lhsT = []
            for j in range(G):
                lt = wp.tile([128, 128], MM_DTYPE, tag=f"lhsT{j}", name=f"lhsT{j}")
                nc.sync.dma_start(out=lt[:, :], in_=w_d[j])
                lhsT.append(lt)

            bias_sb = []
            br = b_d.rearrange("(g p one) -> g p one", p=128, one=1)
            for j in range(G):
                bt = st.tile([128, 1], F32, tag=f"bias{j}", name=f"bias{j}")
                nc.sync.dma_start(out=bt[:, :], in_=br[j])
                bias_sb.append(bt)

            # ---- load x, local BN stats --------------------------------
            xt = []
            spack = st.tile([128, 2 * G], F32, tag="spack", name="spack")
            for j in range(G):
                t = xp.tile([128, f], MM_DTYPE, tag=f"x{j}", name=f"x{j}")
                nc.sync.dma_start(
                    out=t.rearrange("p (n f) -> p n f", n=nl), in_=xr[j]
                )
                xt.append(t)

                st6 = st.tile(
                    [128, (f // FC) * 6], F32, tag=f"st6_{j}", name=f"st6_{j}"
                )
                for k in range(f // FC):
                    nc.vector.bn_stats(
                        out=st6[:, k * 6 : (k + 1) * 6],
                        in_=t[:, k * FC : (k + 1) * FC].bitcast(F32),
                    )
                mv = st.tile([128, 2], F32, tag=f"mv{j}", name=f"mv{j}")
                nc.vector.bn_aggr(
                    out=mv[:, :], in_=st6.rearrange("p (k s) -> p k s", s=3)
                )
                # local sum / sumsq (population of f elems per channel)
                nc.vector.tensor_scalar_mul(
                    spack[:, j : j + 1], mv[:, 0:1], float(f)
                )
                msq = st.tile([128, 1], F32, tag=f"msq{j}", name=f"msq{j}")
                nc.vector.tensor_tensor(
                    msq[:, :], mv[:, 0:1], mv[:, 0:1], ALU.mult
                )
                nc.vector.tensor_tensor(msq[:, :], msq[:, :], mv[:, 1:2], ALU.add)
                nc.vector.tensor_scalar_mul(
                    spack[:, G + j : G + j + 1], msq[:, :], float(f)
                )

            # ---- AllReduce of (sum, sumsq) across cores ----------------
            cc_in = dp.tile([128, 2 * G], F32, tag="ccin", name="ccin")
            cc_out = dp.tile([128, 2 * G], F32, tag="ccout", name="ccout")
            nc.gpsimd.dma_start(out=cc_in[:, :], in_=spack[:, :])
            nc.gpsimd.collective_compute(
                "AllReduce",
                ALU.add,
                replica_groups=[list(range(n_cores))],
                ins=[cc_in.opt()],
                outs=[cc_out.opt()],
            )
            sg = st.tile([128, 2 * G], F32, tag="sg", name="sg")
            nc.gpsimd.dma_start(out=sg[:, :], in_=cc_out[:, :])

            # ---- fold global stats into weights + bias -----------------
            epst = st.tile([128, 1], F32, tag="epst", name="epst")
            nc.vector.memset(epst[:, :], EPS)
            biasp = []
            for j in range(G):
                mean = st.tile([128, 1], F32, tag=f"gmean{j}", name=f"gmean{j}")
                nc.vector.tensor_scalar_mul(mean[:, :], sg[:, j : j + 1], 1.0 / ntot)
                ex2 = st.tile([128, 1], F32, tag=f"gex2{j}", name=f"gex2{j}")
                nc.vector.tensor_scalar_mul(
                    ex2[:, :], sg[:, G + j : G + j + 1], 1.0 / ntot
                )
                msq = st.tile([128, 1], F32, tag=f"gmsq{j}", name=f"gmsq{j}")
                nc.vector.tensor_tensor(msq[:, :], mean[:, :], mean[:, :], ALU.mult)
                var = st.tile([128, 1], F32, tag=f"gvar{j}", name=f"gvar{j}")
                nc.vector.tensor_tensor(var[:, :], ex2[:, :], msq[:, :], ALU.subtract)
                sd = st.tile([128, 1], F32, tag=f"gsd{j}", name=f"gsd{j}")
                nc.scalar.activation(sd[:, :], var[:, :], ACTF.Sqrt, bias=epst[:, :])
                rs = st.tile([128, 1], F32, tag=f"grs{j}", name=f"grs{j}")
                nc.vector.reciprocal(rs[:, :], sd[:, :])
                # scale lhsT rows by rsqrt(var+eps) of the *input* channel
                nc.vector.tensor_scalar_mul(
                    lhsT[j][:, :], lhsT[j][:, :].bitcast(F32), rs[:, :]
                )
                nmean = st.tile([128, 1], F32, tag=f"gnm{j}", name=f"gnm{j}")
                nc.vector.tensor_scalar_mul(nmean[:, :], mean[:, :], -1.0)
                # bias' = bias - W' @ mean   (block-diag matmul with K=128)
                pb = ppb.tile([128, 1], F32, tag="pbias", name=f"pbias{j}")
                nc.tensor.matmul(
                    pb[:, :],
                    lhsT[j][:, :].bitcast(F32),
                    nmean[:, :],
                    start=True,
                    stop=True,
                )
                bp = st.tile([128, 1], F32, tag=f"gbp{j}", name=f"gbp{j}")
                nc.vector.tensor_tensor(bp[:, :], pb[:, :], bias_sb[j][:, :], ALU.add)
                biasp.append(bp)

            # ---- main: grouped conv as block-diag matmul ---------------
            half = f // 2  # stage/DMA granularity: half a group
            for j in range(G):
                stg = sp.tile([128, f], F32, tag="stage", name=f"stage{j}")
                for ch in range(n_chunks):
                    ps = pp.tile([128, FC], F32, tag="ps", name=f"ps{j}_{ch}")
                    nc.tensor.matmul(
                        ps[:, :],
                        lhsT[j][:, :],
                        xt[j][:, ch * FC : (ch + 1) * FC],
                        start=True,
                        stop=True,
                    )
                    if copy_split and (ch % copy_split == copy_split - 1):
                        nc.scalar.activation(
                            stg[:, ch * FC : (ch + 1) * FC],
                            ps[:, :],
                            ACTF.Identity,
                            bias=biasp[j][:, :],
                        )
                    else:
                        nc.vector.tensor_scalar_add(
                            stg[:, ch * FC : (ch + 1) * FC], ps[:, :], biasp[j][:, :]
                        )
                # store both halves as separate DMAs so draining overlaps
                orj = orr[j]  # (128, nl, HW)
                sgr = stg.rearrange("p (n f) -> p n f", n=nl)
                if nl >= 2:
                    nc.sync.dma_start(out=orj[:, : nl // 2, :], in_=sgr[:, : nl // 2, :])
                    nc.sync.dma_start(out=orj[:, nl // 2 :, :], in_=sgr[:, nl // 2 :, :])
                else:
                    nc.sync.dma_start(out=orj, in_=sgr)

    nc.compile()
    return nc


_NC_CACHE: dict = {}


def _get_nc(nl: int, n_cores: int):
    key = (nl, n_cores)
    if key not in _NC_CACHE:
        _NC_CACHE[key] = build_nc(nl=nl, n_cores=n_cores)
    return _NC_CACHE[key]


def make_lhsT_bd(weight: np.ndarray) -> np.ndarray:
    lb = np.zeros((G, 128, 128), dtype=np.float32)
    for c in range(C):
        g, cl = divmod(c, CPG)
        s = cl * D
        lb[g, s : s + D, s : s + D] = weight[c].T  # (i, o)
    return lb


def make_in_maps(x, weight, bias):
    lhsT_bd = make_lhsT_bd(weight)
    return [
        {
            "x_shard": np.ascontiguousarray(
                x[i * NL : (i + 1) * NL].reshape(NL, CD, HW)
            ),
            "lhsT_bd": lhsT_bd,
            "bias": np.ascontiguousarray(bias),
        }
        for i in range(N_CORES)
    ]


def kernel(x: np.ndarray, weight: np.ndarray, bias: np.ndarray) -> np.ndarray:
    assert x.shape == (N_FULL, CD, H, W) and x.dtype == np.float32
    nc = _get_nc(NL, N_CORES)
    in_maps = make_in_maps(x, weight, bias)
    res = run_bass_kernel_spmd(nc, in_maps, core_ids=list(range(N_CORES)))
    out = np.concatenate(
        [res.results[i]["out"].reshape(NL, CD, H, W) for i in range(N_CORES)], axis=0
    )
    return out.astype(np.float32, copy=False)
